# revision 45
# baseline (speedup 1.0000x reference)
"""Trainium2 Bass kernel for an 8-expert top-2 MoE layer (+ shared expert).

Two-phase expert-parallel design, 8 NeuronCores, no device collectives.

Phase 1 (one NEFF, SPMD): token-parallel. Core j owns tokens
[512j, 512j+512): it computes the fp32 router (exact top-2 selection
matching the fp32 reference) for its slice, and the FULL shared-expert FFN
for its slice. Shared-expert matmuls run as fp8-e4m3 DoubleRow with
residual correction (3 terms: x8@q1 + dx8@q1 + x8@q2, residuals encoded in
e4m3 subnormals at the same x64 weight scale) — measurably MORE accurate
than bf16 and 4x the model matmul rate. Outputs: combine weights ct and the
shared output rows (biasless; host adds sb2).

Host dispatch: builds per-expert gather lists from ct, pads to capacity C
(derived from the actual routing, NEFF cache keyed on C), gathers the
pre-quantized x8/dx8 columns per expert.

Phase 2 (one NEFF, SPMD): expert-parallel. Core e runs expert e over its
<=C routed tokens (C = max expert load rounded up to 16, NEFF cached per
C): w1 and w2 as 2-term fp8 DoubleRow (weight residual only; EW1_TERMS=3
re-enables the activation residual), gelu activation emitting h directly
as e4m3, DVE applies the combine weight (with the 1/64 descale folded in)
and returns a compact [C, DIM] bf16 block. Host scatter-adds the expert
blocks, adds sb2 + combine@b2, and reshapes.

All activations keep tokens on the free axis (no on-device transposes);
the host pre-permutes x and weights into SBUF-ready layouts.
"""

import sys

if "/opt/trn_rl_repo" not in sys.path:
    sys.path.insert(0, "/opt/trn_rl_repo")

import numpy as np
import ml_dtypes

DIM = 1024
E = 8
H = 4096
T = 4096  # B*S = 2*2048 tokens
NCORES = 8
P = 128
DKO = DIM // P    # 8 k-subtiles over dim
DKP = DKO // 2    # 4 DoubleRow k-pairs over dim
HKO = H // P      # 32 k-subtiles over hidden
HKP = HKO // 2    # 16 DoubleRow k-pairs over hidden
TSL = T // NCORES  # 512 tokens per core in phase 1
WS = 64.0          # fp8 weight pre-scale
EW1_TERMS = 2      # expert-w1 fp8 terms: 3 = x8@q1 + dx8@q1 + x8@q2, 2 drops dx8

BF16 = ml_dtypes.bfloat16
F8 = ml_dtypes.float8_e4m3  # TRN FP8_EXP4: max +-240

_nc_cache = {}


def _q8(a):
    """Round to e4m3 (fp32 values exactly representable in fp8)."""
    return np.clip(a, -240.0, 240.0).astype(F8)


def _pair_dim(a2d):
    """[K, N] -> [P, K//256, 2, N] DoubleRow pair layout over the K axis."""
    K, N = a2d.shape
    return np.ascontiguousarray(
        a2d.reshape(K // 256, 2, P, N).transpose(2, 0, 1, 3)
    )


def _build_phase1():
    import concourse.mybir as mybir
    import concourse.tile as tile
    from concourse import bacc

    f32 = mybir.dt.float32
    bf16 = mybir.dt.bfloat16
    fp8 = mybir.dt.float8e4
    AF = mybir.ActivationFunctionType
    OP = mybir.AluOpType
    AX = mybir.AxisListType
    DR = mybir.MatmulPerfMode.DoubleRow
    nc = bacc.Bacc("TRN2", target_bir_lowering=False, debug=False,
                   num_devices=NCORES)

    x32 = nc.dram_tensor("x32", [P, DKO, TSL], f32, kind="ExternalInput")
    x8 = nc.dram_tensor("x8", [P, DKP, 2, TSL], fp8, kind="ExternalInput")
    dx8 = nc.dram_tensor("dx8", [P, DKP, 2, TSL], fp8, kind="ExternalInput")
    rwp = nc.dram_tensor("rwp", [P, DKO, E], f32, kind="ExternalInput")
    rb = nc.dram_tensor("rb", [P, E], f32, kind="ExternalInput")
    sq1 = nc.dram_tensor("sq1", [P, DKP, 2, H], fp8, kind="ExternalInput")
    sq2 = nc.dram_tensor("sq2", [P, DKP, 2, H], fp8, kind="ExternalInput")
    sb1c = nc.dram_tensor("sb1c", [P, HKO], f32, kind="ExternalInput")
    t1q = nc.dram_tensor("t1q", [P, HKP, 2, DIM], fp8, kind="ExternalInput")
    t2q = nc.dram_tensor("t2q", [P, HKP, 2, DIM], fp8, kind="ExternalInput")
    ct_out = nc.dram_tensor("ct", [TSL, E], f32, kind="ExternalOutput")
    ysh = nc.dram_tensor("ysh", [TSL, DIM], bf16, kind="ExternalOutput")

    NT4 = TSL // P  # 4 token tiles

    with tile.TileContext(nc) as tc:
        with (
            tc.tile_pool(name="const", bufs=1) as const,
            tc.tile_pool(name="wpool", bufs=1) as wpool,
            tc.tile_pool(name="xp", bufs=1) as xp,
            tc.tile_pool(name="rt", bufs=2) as rt,
            tc.tile_pool(name="hbp", bufs=3) as hbp,
            tc.tile_pool(name="h8p", bufs=1) as h8p,
            tc.tile_pool(name="yp", bufs=2) as yp,
            tc.tile_pool(name="rps", bufs=1, space="PSUM") as rps,
            tc.tile_pool(name="p1", bufs=3, space="PSUM") as p1p,
            tc.tile_pool(name="p2", bufs=2, space="PSUM") as p2p,
        ):
            # --- inputs: x first (router + w1 can start), then weights.
            # Chunked + interleaved so the first consumers of each pair of
            # tensors unblock as early as possible.
            # SP queue: x tensors (router + w1 critical path). Pool queue:
            # the 16MB weight stream — two DGE queues run in parallel.
            x8_sb = xp.tile([P, DKP, 2, TSL], fp8)
            dx8_sb = xp.tile([P, DKP, 2, TSL], fp8)
            sq1_sb = wpool.tile([P, DKP, 2, H], fp8)
            sq2_sb = wpool.tile([P, DKP, 2, H], fp8)
            nc.sync.dma_start(x8_sb, x8[:, :, :, :])
            HCHUNK = 512
            nc.sync.dma_start(sq1_sb[:, :, :, 0:P], sq1[:, :, :, 0:P])
            nc.sync.dma_start(sq2_sb[:, :, :, 0:P], sq2[:, :, :, 0:P])
            nc.sync.dma_start(dx8_sb, dx8[:, :, :, :])
            nc.sync.dma_start(sq1_sb[:, :, :, P:HCHUNK], sq1[:, :, :, P:HCHUNK])
            nc.sync.dma_start(sq2_sb[:, :, :, P:HCHUNK], sq2[:, :, :, P:HCHUNK])
            sb1c_sb = const.tile([P, HKO], f32)
            nc.sync.dma_start(sb1c_sb, sb1c[:, :])
            for h0 in range(HCHUNK, H, HCHUNK):
                nc.sync.dma_start(sq1_sb[:, :, :, h0:h0 + HCHUNK],
                                  sq1[:, :, :, h0:h0 + HCHUNK])
                nc.sync.dma_start(sq2_sb[:, :, :, h0:h0 + HCHUNK],
                                  sq2[:, :, :, h0:h0 + HCHUNK])
            t1q_sb = wpool.tile([P, HKP, 2, DIM], fp8)
            t2q_sb = wpool.tile([P, HKP, 2, DIM], fp8)
            for k0 in range(0, HKP, 4):
                nc.sync.dma_start(t1q_sb[:, k0:k0 + 4, :, :],
                                  t1q[:, k0:k0 + 4, :, :])
                nc.sync.dma_start(t2q_sb[:, k0:k0 + 4, :, :],
                                  t2q[:, k0:k0 + 4, :, :])
            x32_sb = xp.tile([P, DKO, TSL], f32)
            nc.sync.dma_start(x32_sb, x32[:, :, :])
            rwp_sb = const.tile([P, DKO, E], f32)
            nc.sync.dma_start(rwp_sb, rwp[:, :, :])
            rb_sb = const.tile([P, E], f32)
            nc.sync.dma_start(rb_sb, rb[:, :])

            # --- shared expert w1: h = gelu(x@sw1/1 ... psum is 64x) ---
            # 3-term fp8 DR: x8@q1 + x8@q2 + dx8@q1; h8/dh8 for w2.
            h8_sb = h8p.tile([P, HKO, TSL], fp8, tag="h8")
            dh8_sb = h8p.tile([P, HKO, TSL], fp8, tag="dh8")
            for hm in range(HKO):
                ps = p1p.tile([P, TSL], f32, tag="ps1")
                for kop in range(DKP):
                    nc.tensor.matmul(ps, sq1_sb[:, kop, :, hm * P:(hm + 1) * P],
                                     x8_sb[:, kop, :, :],
                                     start=(kop == 0), stop=False,
                                     perf_mode=DR)
                    nc.tensor.matmul(ps, sq2_sb[:, kop, :, hm * P:(hm + 1) * P],
                                     x8_sb[:, kop, :, :],
                                     start=False, stop=False, perf_mode=DR)
                for kop in range(DKP):
                    nc.tensor.matmul(ps, sq1_sb[:, kop, :, hm * P:(hm + 1) * P],
                                     dx8_sb[:, kop, :, :],
                                     start=False, stop=(kop == DKP - 1),
                                     perf_mode=DR)
                hb = hbp.tile([P, TSL], bf16, tag="hb")
                nc.scalar.activation(hb, ps, AF.Gelu,
                                     bias=sb1c_sb[:, hm:hm + 1],
                                     scale=1.0 / WS)
                nc.vector.tensor_copy(h8_sb[:, hm, :], hb)
                nc.vector.tensor_tensor(dh8_sb[:, hm, :], hb, h8_sb[:, hm, :],
                                        OP.subtract)

            # --- router: fp32 logits, softmax, exact top-2 (as reference).
            # Emitted after the w1 stage so its x32 input streams in behind
            # the fp8 activations without stalling the PE at phase start. ---
            lg4 = rt.tile([P, NT4, E], f32, tag="lg4")
            for t4 in range(NT4):
                pl = rps.tile([P, E], f32, tag="pl")
                for ko in range(DKO):
                    nc.tensor.matmul(pl, x32_sb[:, ko, t4 * P:(t4 + 1) * P],
                                     rwp_sb[:, ko, :],
                                     start=(ko == 0), stop=(ko == DKO - 1))
                nc.vector.tensor_add(lg4[:, t4, :], pl, rb_sb)
            mx4 = rt.tile([P, NT4, 1], f32, tag="mx4")
            nc.vector.reduce_max(mx4, lg4, axis=AX.X)
            lgs = rt.tile([P, NT4, E], f32, tag="lgs")
            nc.vector.tensor_sub(lgs, lg4, mx4.to_broadcast((P, NT4, E)))
            ex4 = rt.tile([P, NT4, E], f32, tag="ex4")
            nc.scalar.activation(ex4, lgs, AF.Exp)
            sm4 = rt.tile([P, NT4, 1], f32, tag="sm4")
            nc.vector.reduce_sum(sm4, ex4, axis=AX.X)
            rc4 = rt.tile([P, NT4, 1], f32, tag="rc4")
            nc.vector.reciprocal(rc4, sm4)
            ge1 = rt.tile([P, NT4, E], f32, tag="ge1")
            nc.vector.tensor_scalar(ge1, lgs, 0.0, 1e30, OP.is_ge, OP.mult)
            lm4 = rt.tile([P, NT4, E], f32, tag="lm4")
            nc.vector.tensor_sub(lm4, lgs, ge1)
            m24 = rt.tile([P, NT4, 1], f32, tag="m24")
            nc.vector.reduce_max(m24, lm4, axis=AX.X)
            msk4 = rt.tile([P, NT4, E], f32, tag="msk4")
            nc.vector.tensor_tensor(msk4, lgs, m24.to_broadcast((P, NT4, E)),
                                    OP.is_ge)
            pw4 = rt.tile([P, NT4, E], f32, tag="pw4")
            nc.vector.tensor_mul(pw4, ex4, msk4)
            nc.vector.tensor_tensor(pw4, pw4, rc4.to_broadcast((P, NT4, E)),
                                    OP.mult)
            nc.sync.dma_start(
                ct_out[:, :].rearrange("(t4 p) e -> p t4 e", p=P), pw4)

            # --- shared expert w2: y = (h@sw2) (64x in psum; DVE descales) ---
            for t4 in range(NT4):
                tsl = slice(t4 * P, (t4 + 1) * P)
                y_sb = yp.tile([P, DIM], bf16, tag="y_sb")
                for half in range(2):
                    dsl = slice(half * 512, half * 512 + 512)
                    ps2 = p2p.tile([P, 512], f32, tag=f"ps2_{half}")
                    for kp in range(HKP):
                        h8pair = h8_sb[:, 2 * kp:2 * kp + 2, tsl]
                        nc.tensor.matmul(ps2, h8pair,
                                         t1q_sb[:, kp, :, dsl],
                                         start=(kp == 0), stop=False,
                                         perf_mode=DR)
                        nc.tensor.matmul(ps2, dh8_sb[:, 2 * kp:2 * kp + 2, tsl],
                                         t1q_sb[:, kp, :, dsl],
                                         start=False, stop=False, perf_mode=DR)
                        nc.tensor.matmul(ps2, h8pair,
                                         t2q_sb[:, kp, :, dsl],
                                         start=False, stop=(kp == HKP - 1),
                                         perf_mode=DR)
                    nc.vector.tensor_scalar_mul(y_sb[:, dsl], ps2, 1.0 / WS)
                    nc.sync.dma_start(ysh[t4 * P:(t4 + 1) * P, dsl],
                                      y_sb[:, dsl])

    nc.finalize()
    return nc


def _build_phase2(C):
    import concourse.mybir as mybir
    import concourse.tile as tile
    from concourse import bacc

    f32 = mybir.dt.float32
    bf16 = mybir.dt.bfloat16
    fp8 = mybir.dt.float8e4
    AF = mybir.ActivationFunctionType
    OP = mybir.AluOpType
    DR = mybir.MatmulPerfMode.DoubleRow
    nc = bacc.Bacc("TRN2", target_bir_lowering=False, debug=False,
                   num_devices=NCORES)

    CT = (C + P - 1) // P  # token tiles (last may be partial)
    # token slices of <=512 for the w1 stage
    slices = []
    c0 = 0
    while c0 < C:
        w = min(512, C - c0)
        slices.append((c0, w))
        c0 += w

    xg8 = nc.dram_tensor("xg8", [P, DKP, 2, C], fp8, kind="ExternalInput")
    dxg8 = (nc.dram_tensor("dxg8", [P, DKP, 2, C], fp8, kind="ExternalInput")
            if EW1_TERMS == 3 else None)
    w1q1 = nc.dram_tensor("w1q1", [P, DKP, 2, H], fp8, kind="ExternalInput")
    w1q2 = nc.dram_tensor("w1q2", [P, DKP, 2, H], fp8, kind="ExternalInput")
    b1c = nc.dram_tensor("b1c", [P, HKO], f32, kind="ExternalInput")
    w2q1 = nc.dram_tensor("w2q1", [P, HKP, 2, DIM], fp8, kind="ExternalInput")
    w2q2 = nc.dram_tensor("w2q2", [P, HKP, 2, DIM], fp8, kind="ExternalInput")
    ceg = nc.dram_tensor("ceg", [P, CT], f32, kind="ExternalInput")
    eout = nc.dram_tensor("eout", [C, DIM], bf16, kind="ExternalOutput")

    with tile.TileContext(nc) as tc:
        with (
            tc.tile_pool(name="const", bufs=1) as const,
            tc.tile_pool(name="wpool", bufs=1) as wpool,
            tc.tile_pool(name="h8p", bufs=2) as h8p,
            tc.tile_pool(name="yp", bufs=2) as yp,
            tc.tile_pool(name="p1", bufs=3, space="PSUM") as p1p,
            tc.tile_pool(name="p2", bufs=2, space="PSUM") as p2p,
        ):
            # inputs: slice-0 x + first w1 chunks first so compute starts
            # early; later x slices and the w2 weights follow.
            xg8_sb = wpool.tile([P, DKP, 2, C], fp8)
            dxg8_sb = wpool.tile([P, DKP, 2, C], fp8) if EW1_TERMS == 3 else None
            w1q1_sb = wpool.tile([P, DKP, 2, H], fp8)
            w1q2_sb = wpool.tile([P, DKP, 2, H], fp8)
            s0w, Ww = slices[0]
            nc.sync.dma_start(xg8_sb[:, :, :, s0w:s0w + Ww],
                              xg8[:, :, :, s0w:s0w + Ww])
            if EW1_TERMS == 3:
                nc.sync.dma_start(dxg8_sb[:, :, :, s0w:s0w + Ww],
                                  dxg8[:, :, :, s0w:s0w + Ww])
            HCHUNK = 512
            nc.sync.dma_start(w1q1_sb[:, :, :, 0:P], w1q1[:, :, :, 0:P])
            nc.sync.dma_start(w1q2_sb[:, :, :, 0:P], w1q2[:, :, :, 0:P])
            b1c_sb = const.tile([P, HKO], f32)
            nc.sync.dma_start(b1c_sb, b1c[:, :])
            ceg_sb = const.tile([P, CT], f32)
            nc.sync.dma_start(ceg_sb, ceg[:, :])
            nc.sync.dma_start(w1q1_sb[:, :, :, P:HCHUNK], w1q1[:, :, :, P:HCHUNK])
            nc.sync.dma_start(w1q2_sb[:, :, :, P:HCHUNK], w1q2[:, :, :, P:HCHUNK])
            for h0 in range(HCHUNK, H, HCHUNK):
                nc.sync.dma_start(w1q1_sb[:, :, :, h0:h0 + HCHUNK],
                                  w1q1[:, :, :, h0:h0 + HCHUNK])
                nc.sync.dma_start(w1q2_sb[:, :, :, h0:h0 + HCHUNK],
                                  w1q2[:, :, :, h0:h0 + HCHUNK])
            w2q1_sb = wpool.tile([P, HKP, 2, DIM], fp8)
            w2q2_sb = wpool.tile([P, HKP, 2, DIM], fp8)
            if len(slices) > 1:
                s1, W1 = slices[1]
                nc.sync.dma_start(xg8_sb[:, :, :, s1:s1 + W1],
                                  xg8[:, :, :, s1:s1 + W1])
                if EW1_TERMS == 3:
                    nc.sync.dma_start(dxg8_sb[:, :, :, s1:s1 + W1],
                                      dxg8[:, :, :, s1:s1 + W1])
            for k0 in range(0, HKP, 4):
                nc.sync.dma_start(w2q1_sb[:, k0:k0 + 4, :, :],
                                  w2q1[:, k0:k0 + 4, :, :])
                nc.sync.dma_start(w2q2_sb[:, k0:k0 + 4, :, :],
                                  w2q2[:, k0:k0 + 4, :, :])
            for s0, W in slices[2:]:
                nc.sync.dma_start(xg8_sb[:, :, :, s0:s0 + W],
                                  xg8[:, :, :, s0:s0 + W])
                if EW1_TERMS == 3:
                    nc.sync.dma_start(dxg8_sb[:, :, :, s0:s0 + W],
                                      dxg8[:, :, :, s0:s0 + W])

            for s0, W in slices:
                # --- w1 stage: h8 = e4m3(gelu(psum/64 + b1)) ---
                h8_sb = h8p.tile([P, HKO, 512], fp8, tag="h8")
                for hm in range(HKO):
                    ps = p1p.tile([P, 512], f32, tag="ps1")
                    for kop in range(DKP):
                        q1w = w1q1_sb[:, kop, :, hm * P:(hm + 1) * P]
                        nc.tensor.matmul(ps[:, :W], q1w,
                                         xg8_sb[:, kop, :, s0:s0 + W],
                                         start=(kop == 0), stop=False,
                                         perf_mode=DR)
                        if EW1_TERMS == 3:
                            nc.tensor.matmul(ps[:, :W], q1w,
                                             dxg8_sb[:, kop, :, s0:s0 + W],
                                             start=False, stop=False,
                                             perf_mode=DR)
                        nc.tensor.matmul(ps[:, :W],
                                         w1q2_sb[:, kop, :, hm * P:(hm + 1) * P],
                                         xg8_sb[:, kop, :, s0:s0 + W],
                                         start=False, stop=(kop == DKP - 1),
                                         perf_mode=DR)
                    nc.scalar.activation(h8_sb[:, hm, :W], ps[:, :W], AF.Gelu,
                                         bias=b1c_sb[:, hm:hm + 1],
                                         scale=1.0 / WS)
                # --- w2 stage: y = ce/64 * (h8 @ (w2q1+w2q2)) ---
                for tt in range((W + P - 1) // P):
                    gtt = s0 // P + tt
                    TW = min(P, W - tt * P)
                    tsl = slice(tt * P, tt * P + TW)
                    y_sb = yp.tile([P, DIM], bf16, tag="y_sb")
                    for half in range(2):
                        dsl = slice(half * 512, half * 512 + 512)
                        ps2 = p2p.tile([P, 512], f32, tag=f"ps2_{half}")
                        for kp in range(HKP):
                            h8pair = h8_sb[:, 2 * kp:2 * kp + 2, tsl]
                            nc.tensor.matmul(ps2[:TW, :], h8pair,
                                             w2q1_sb[:, kp, :, dsl],
                                             start=(kp == 0), stop=False,
                                             perf_mode=DR)
                            nc.tensor.matmul(ps2[:TW, :], h8pair,
                                             w2q2_sb[:, kp, :, dsl],
                                             start=False, stop=(kp == HKP - 1),
                                             perf_mode=DR)
                        nc.vector.tensor_tensor(
                            y_sb[:TW, dsl], ps2[:TW, :],
                            ceg_sb[:TW, gtt:gtt + 1].to_broadcast((TW, 512)),
                            OP.mult)
                    nc.sync.dma_start(eout[s0 + tt * P:s0 + tt * P + TW, :],
                                      y_sb[:TW, :])

    nc.finalize()
    return nc


def _get(name, builder):
    if name not in _nc_cache:
        _nc_cache[name] = builder()
    return _nc_cache[name]


def _prep_phase1(x, router_w, router_b, sw1, sb1, sw2):
    xt = np.ascontiguousarray(x.reshape(T, DIM).astype(np.float32).T)  # [DIM,T]
    x8f = _q8(xt)
    dx8f = _q8(xt - x8f.astype(np.float32))
    xt32p = np.ascontiguousarray(xt.reshape(DKO, P, T).transpose(1, 0, 2))
    x8p = _pair_dim(x8f)    # [P, DKP, 2, T]
    dx8p = _pair_dim(dx8f)
    rwp = np.ascontiguousarray(
        router_w.astype(np.float32).reshape(DKO, P, E).transpose(1, 0, 2))
    rb = np.ascontiguousarray(
        np.tile(router_b.astype(np.float32)[None, :], (P, 1)))
    sw1s = sw1.astype(np.float32) * WS
    sq1 = _q8(sw1s)
    sq2 = _q8(sw1s - sq1.astype(np.float32))
    sq1p = _pair_dim(sq1)
    sq2p = _pair_dim(sq2)
    sb1c = np.ascontiguousarray(sb1.astype(np.float32).reshape(HKO, P).T)
    sw2s = sw2.astype(np.float32) * WS
    t1 = _q8(sw2s)
    t2 = _q8(sw2s - t1.astype(np.float32))
    t1p = _pair_dim(t1)
    t2p = _pair_dim(t2)
    maps = []
    for j in range(NCORES):
        ts = slice(j * TSL, (j + 1) * TSL)
        maps.append(dict(
            x32=np.ascontiguousarray(xt32p[:, :, ts]),
            x8=np.ascontiguousarray(x8p[:, :, :, ts]),
            dx8=np.ascontiguousarray(dx8p[:, :, :, ts]),
            rwp=rwp, rb=rb, sq1=sq1p, sq2=sq2p, sb1c=sb1c,
            t1q=t1p, t2q=t2p))
    return maps, x8f, dx8f


def _prep_phase2(ct, x8f, dx8f, w1, b1, w2, C):
    CT = (C + P - 1) // P
    maps = []
    sels = []
    for e in range(NCORES):
        sel = np.nonzero(ct[:, e])[0].astype(np.int64)
        sels.append(sel)
        npad = C - len(sel)
        selp = np.concatenate([sel, np.zeros(npad, np.int64)])
        cev = np.concatenate([ct[sel, e].astype(np.float32),
                              np.zeros(CT * P - len(sel), np.float32)]) / WS
        w1s = w1[e].astype(np.float32) * WS
        q1 = _q8(w1s)
        q2 = _q8(w1s - q1.astype(np.float32))
        w2s = w2[e].astype(np.float32) * WS
        u1 = _q8(w2s)
        u2 = _q8(w2s - u1.astype(np.float32))
        m = dict(
            xg8=_pair_dim(x8f[:, selp]),
            w1q1=_pair_dim(q1), w1q2=_pair_dim(q2),
            b1c=np.ascontiguousarray(b1[e].astype(np.float32).reshape(HKO, P).T),
            w2q1=_pair_dim(u1), w2q2=_pair_dim(u2),
            ceg=np.ascontiguousarray(cev.reshape(CT, P).T))
        if EW1_TERMS == 3:
            m["dxg8"] = _pair_dim(dx8f[:, selp])
        maps.append(m)
    return maps, sels


def _run_spmd(nc, in_maps):
    from concourse.bass_utils import run_bass_kernel_spmd
    return run_bass_kernel_spmd(nc, in_maps, core_ids=list(range(NCORES)))


def kernel(x, router_w, router_b, w1, b1, w2, b2, sw1, sb1, sw2, sb2):
    maps1, x8f, dx8f = _prep_phase1(x, router_w, router_b, sw1, sb1, sw2)
    res1 = _run_spmd(_get("p1", _build_phase1), maps1)

    ct = np.concatenate([r["ct"] for r in res1.results], axis=0)  # [T, E]
    out = np.concatenate([r["ysh"] for r in res1.results],
                         axis=0).astype(np.float32)               # [T, DIM]
    out += sb2.astype(np.float32)[None, :]
    out += ct @ b2.astype(np.float32)  # combine-weighted expert biases

    nmax = int(max((ct[:, e] != 0).sum() for e in range(E)))
    C = max(P, (nmax + 15) // 16 * 16)
    maps2, sels = _prep_phase2(ct, x8f, dx8f, w1, b1, w2, C)
    res2 = _run_spmd(_get(f"p2_{C}", lambda: _build_phase2(C)), maps2)
    for e in range(NCORES):
        n = len(sels[e])
        out[sels[e]] += res2.results[e]["eout"][:n].astype(np.float32)
    return out.reshape(2, 2048, DIM)
